# revision 4
# baseline (speedup 1.0000x reference)
"""Trainium2 Bass kernel for DiffeqSolver (fixed-grid RK4 over a tanh-MLP ODE).

reference:
  f(y) = tanh(y @ W1 + b1) @ W2 + b2        y: [B, D], W1: [D, H], W2: [H, D]
  63 RK4 steps over time_steps[64]; output pred_y [T=64, B=1024, D=512].

Strategy:
  - Data-parallel over batch: 8 cores x 128 rows each. No collectives.
  - All-feature-major on device: state y^T with D on partitions (4 chunks of
    128), batch (128) on the free dim. Both matmuls then use the weights as
    the stationary operand (lhsT) directly -- no activation transposes ever.
      h^T[m] = sum_c W1[c,m]^T @ u^T[c]     (32 matmuls, N=128)
      z^T[j] = sum_k W2[k,j]^T @ g^T[k]     (32 matmuls, N=128)
  - Matmul operands in fp16 (1 cycle/row on PE; fp32 would be 4). PSUM
    accumulation and the RK4 state/combines stay fp32. Empirically this
    yields ~1e-4 global relative error on this problem.
  - dt values and weight layouts are specialized on the host per call.
  - The RK4 combine uses an incremental p-chain (p_i = p_{i-1} + w_i dt/6 k_i)
    so the step boundary only waits on the last stage's z.
  - Output is DMA'd feature-major straight from the state tile (contiguous,
    no PE transposes); the host undoes the transpose when assembling pred_y.
"""

import os
import sys

import numpy as np

if "/opt/trn_rl_repo" not in sys.path:
    sys.path.insert(0, "/opt/trn_rl_repo")

import concourse.bass as bass
import concourse.mybir as mybir
import concourse.tile as tile
from concourse import bacc
from concourse.bass_utils import run_bass_kernel_spmd

B, D, H, T = 1024, 512, 1024, 64
NCORES = 8
BP = B // NCORES          # 128 batch rows per core
DC = D // 128             # 4 D-chunks
HC = H // 128             # 8 H-chunks
NSTEP = T - 1

F32 = mybir.dt.float32
F16 = mybir.dt.float16
F8 = mybir.dt.float8e4

# fp8 path: weights are pre-scaled by a power of two on the host so the
# bulk of the distribution sits in e4m3 normal range (min normal 2^-6;
# unscaled W1/W2 entries have std 1/sqrt(D) ~ 0.03-0.04). The /scale is
# folded into the tanh's pre-scale (MM1) and the RK4 stage constants
# (MM2) -- zero runtime cost.
W1SCALE = 64.0
W2SCALE = 64.0

# "f8": e4m3 + DoubleRow matmuls (2 K-rows/cycle). "f16": original path.
MM_MODE = os.environ.get("MM_MODE", "f8")


def _build_program(dts, has_b1, has_b2, mm_dtype=F16, compile=True, reps=1, timing=False, ablate=(), fm_out=True):
    """Trace + compile the per-core SPMD program. dts: list of python floats.

    timing=True: outputs go to internal DRAM (not transferred) and the body
    repeats `reps` times in a HW loop -- for differential wall-clock timing."""
    nsteps = len(dts)
    nc = bacc.Bacc(
        "TRN2",
        target_bir_lowering=False,
        debug=False,
        enable_asserts=True,
        num_devices=NCORES,
    )

    w1r = nc.dram_tensor("w1r", [128, DC * HC * 128], mm_dtype, kind="ExternalInput")
    w2r = nc.dram_tensor("w2r", [128, HC * DC * 128], mm_dtype, kind="ExternalInput")
    ident = nc.dram_tensor("ident", [128, 128], F32, kind="ExternalInput")
    fp32d = nc.dram_tensor("fp32d", [128, D], F32, kind="ExternalInput")
    fp16d = nc.dram_tensor("fp16d", [128, D], mm_dtype, kind="ExternalInput")
    if has_b1:
        b1d = nc.dram_tensor("b1c", [128, HC], F32, kind="ExternalInput")
    if has_b2:
        b2d = nc.dram_tensor("b2c", [128, DC], F32, kind="ExternalInput")
    if timing:
        tout_d = nc.dram_tensor("tout", [128, 4], F32, kind="ExternalOutput")
    else:
        out_d = nc.dram_tensor("yout", [nsteps, 128, D], F32, kind="ExternalOutput")

    AF = mybir.ActivationFunctionType
    OP = mybir.AluOpType

    with tile.TileContext(nc) as tc, tc.tile_pool(name="persist", bufs=1) as persist:
        # ---- persistent tiles -------------------------------------------
        w1sb = persist.tile([128, DC * HC * 128], mm_dtype, tag="w1sb", name="w1sb")
        w2sb = persist.tile([128, HC * DC * 128], mm_dtype, tag="w2sb", name="w2sb")
        idsb = persist.tile([128, 128], F32, tag="idsb", name="idsb")
        yT = persist.tile([128, D], F32, tag="yT", name="yT")      # fp32 state
        u0 = persist.tile([128, D], mm_dtype, tag="u0", name="u0")
        if "tanh" in ablate:
            u0big = persist.tile([128, H], mm_dtype, tag="u0big", name="u0big")
            touch = persist.tile([128, 64], F32, tag="touch", name="touch")
            nc.gpsimd.memset(u0big[:], 0.01)
        elif "dve" in ablate:
            touch = persist.tile([128, 64], F32, tag="touch", name="touch")
        if has_b1:
            b1sb = persist.tile([128, HC], F32, tag="b1sb", name="b1sb")
        if has_b2:
            b2sb = persist.tile([128, DC], F32, tag="b2sb", name="b2sb")

        nc.sync.dma_start(w1sb[:], w1r[:])
        nc.sync.dma_start(w2sb[:], w2r[:])
        nc.sync.dma_start(idsb[:], ident[:])
        nc.sync.dma_start(yT[:], fp32d[:])
        nc.sync.dma_start(u0[:], fp16d[:])
        if has_b1:
            nc.sync.dma_start(b1sb[:], b1d[:])
        if has_b2:
            nc.sync.dma_start(b2sb[:], b2d[:])

        with (
            tc.tile_pool(name="dram", bufs=1, space="DRAM") as dram_pool,
            tc.tile_pool(name="hps", bufs=2, space="PSUM") as hps_pool,
            tc.tile_pool(name="zps", bufs=4 if "zps4" in ablate else 3, space="PSUM") as zps_pool,
            tc.tile_pool(name="ybm", bufs=1, space="PSUM") as ybm_pool,
            tc.tile_pool(name="upool", bufs=3 if "sbuf3" in ablate else 2) as upool,
            tc.tile_pool(name="ppool", bufs=3 if "sbuf3" in ablate else 2) as ppool,
            tc.tile_pool(name="gpool", bufs=3 if "sbuf3" in ablate else 2) as gpool,
            tc.tile_pool(name="kts", bufs=2) as ktpool,
            tc.tile_pool(name="yo", bufs=3) as yopool,
        ):
            def w1chunk(c, m):
                s = (c * HC + m) * 128
                return w1sb[:, s : s + 128]

            def w2chunk(k, j):
                s = (k * DC + j) * 128
                return w2sb[:, s : s + 128]

            def f_eval(u16):
                """u16: fp16 [128, D] feature-major eval point.
                Returns zT psum tile [128, D] fp32 (= f(u) - b2, feature-major)."""
                hps = hps_pool.tile([128, H], F32, tag="hps")
                for m in range(HC):
                    om = hps[:, m * 128 : (m + 1) * 128]
                    for c in range(DC):
                        nc.tensor.matmul(
                            om,
                            w1chunk(c, m),
                            u16[:, c * 128 : (c + 1) * 128],
                            start=(c == 0),
                            stop=(c == DC - 1),
                        )
                gt = gpool.tile([128, H], mm_dtype, tag="gt")
                if "tanh" in ablate:
                    # timing-ablation: break the MM1->ACT->MM2 dependency; MM2
                    # streams from a static tile; touch hps so tiles release.
                    nc.vector.tensor_copy(touch[:, 0:8], hps[:, 0:1024:128])
                    gt = u0big
                elif has_b1:
                    for m in range(HC):
                        sl = slice(m * 128, (m + 1) * 128)
                        nc.scalar.activation(
                            gt[:, sl], hps[:, sl], AF.Tanh, bias=b1sb[:, m : m + 1]
                        )
                else:
                    # bank0 whole, bank1 split in two: MM2's last k-chunks
                    # wait on a 256-wide ACT op instead of 512 (A/B-measured
                    # win together with zps bufs=3)
                    nc.scalar.activation(gt[:, :512], hps[:, :512], AF.Tanh)
                    nc.scalar.activation(gt[:, 512:768], hps[:, 512:768], AF.Tanh)
                    nc.scalar.activation(gt[:, 768:], hps[:, 768:], AF.Tanh)
                if "dve" in ablate:
                    # timing-ablation: MM1 of every eval streams from u0
                    # (vector STT chain off the critical path entirely)
                    pass
                zps = zps_pool.tile([128, D], F32, tag="zps")
                for j in range(DC):
                    oj = zps[:, j * 128 : (j + 1) * 128]
                    for k in range(HC):
                        nc.tensor.matmul(
                            oj,
                            w2chunk(k, j),
                            gt[:, k * 128 : (k + 1) * 128],
                            start=(k == 0),
                            stop=(k == HC - 1),
                        )
                return zps

            if timing:
                out_d = dram_pool.tile([nsteps, 128, D], F32, name="out_i")

            from contextlib import nullcontext

            def emit_output(t):
                if "output" in ablate:
                    return
                if fm_out:
                    # feature-major dump: contiguous DMA straight from the
                    # state tile; the host undoes the transpose. Saves the PE
                    # transposes + PSUM evacuation entirely.
                    nc.sync.dma_start(out_d[t], yT[:])
                    return
                # batch-major output for step t (reads yT as of end of step t):
                # 4 PE transposes -> PSUM, evacuate, DMA out. Emitted lazily
                # during step t+1 so it never stalls the PE at the boundary.
                ybm = ybm_pool.tile([128, D], F32, tag="ybm")
                for c in range(DC):
                    sl = slice(c * 128, (c + 1) * 128)
                    nc.tensor.transpose(ybm[:, sl], yT[:, sl], idsb[:])
                yo = yopool.tile([128, D], F32, tag="yo")
                nc.scalar.copy(yo[:], ybm[:])
                nc.sync.dma_start(out_d[t], yo[:])

            loop_ctx = tc.For_i(0, reps, 1) if reps > 1 else nullcontext()
            u_cur = u0
            with loop_ctx:
                for t in range(nsteps):
                    dt = dts[t]
                    # RK4: u_{i+1} = y + c_i k_i;  y' = y + dt/6 sum w_i k_i.
                    # Incremental p-chain: p_i = p_{i-1} + (w_i dt/6) k_i with
                    # p_0 = y, so the boundary only waits on the last z.
                    stage_c = [dt * 0.5, dt * 0.5, dt]
                    pw = [dt / 6.0, dt / 3.0, dt / 3.0, dt / 6.0]
                    p_prev = yT
                    for i in range(4):
                        zps = f_eval(u_cur)
                        if "dve" in ablate:
                            nc.vector.tensor_copy(touch[:, 8:12], zps[:, 0:512:128])
                            continue
                        if has_b2:
                            kt = ktpool.tile([128, D], F32, tag="kt")
                            for j in range(DC):
                                sl = slice(j * 128, (j + 1) * 128)
                                nc.vector.tensor_scalar_add(
                                    kt[:, sl], zps[:, sl], b2sb[:, j : j + 1]
                                )
                            ksrc = kt
                        else:
                            ksrc = zps
                        if i < 3:
                            un = upool.tile([128, D], mm_dtype, tag="un")
                            if "ustt2" in ablate:
                                # first half reads z chunks 0-1 (ready at 50%
                                # of MM2, j-outer) -> runs under MM2's tail;
                                # only the 256-wide second half is exposed.
                                nc.vector.scalar_tensor_tensor(
                                    un[:, 0:256], ksrc[:, 0:256], stage_c[i], yT[:, 0:256], OP.mult, OP.add
                                )
                                nc.vector.scalar_tensor_tensor(
                                    un[:, 256:512], ksrc[:, 256:512], stage_c[i], yT[:, 256:512], OP.mult, OP.add
                                )
                            else:
                                nc.vector.scalar_tensor_tensor(
                                    un[:], ksrc[:], stage_c[i], yT[:], OP.mult, OP.add
                                )
                            u_cur = un
                            pn = ppool.tile([128, D], F32, tag="pn")
                            nc.vector.scalar_tensor_tensor(
                                pn[:], ksrc[:], pw[i], p_prev[:], OP.mult, OP.add
                            )
                            p_prev = pn
                        else:
                            # y_{t+1} = p3 + (dt/6) k4: fp16 for the next
                            # step's first eval point (critical path) first,
                            # then the fp32 state update.
                            if t < nsteps - 1 or timing:
                                un = upool.tile([128, D], mm_dtype, tag="un")
                                if "ustt2" in ablate:
                                    nc.vector.scalar_tensor_tensor(
                                        un[:, 0:256], ksrc[:, 0:256], pw[i], p_prev[:, 0:256], OP.mult, OP.add
                                    )
                                    nc.vector.scalar_tensor_tensor(
                                        un[:, 256:512], ksrc[:, 256:512], pw[i], p_prev[:, 256:512], OP.mult, OP.add
                                    )
                                else:
                                    nc.vector.scalar_tensor_tensor(
                                        un[:], ksrc[:], pw[i], p_prev[:], OP.mult, OP.add
                                    )
                                u_cur = un
                            nc.vector.scalar_tensor_tensor(
                                yT[:], ksrc[:], pw[i], p_prev[:], OP.mult, OP.add
                            )
                        if i == 0 and t > 0:
                            # step t-1's output block, emitted mid-step so the
                            # PE transposes hide behind eval-1 matmuls (yT
                            # still holds y_t here; it's rewritten at i==3).
                            emit_output(t - 1)
                emit_output(nsteps - 1)

            if timing:
                dyo = yopool.tile([128, 4], F32, tag="dyo")
                nc.vector.tensor_copy(dyo[:], yT[:, 0:4])
                nc.sync.dma_start(tout_d[:], dyo[:])

    if compile:
        nc.compile()
    return nc


_cache = {}


def _host_in_maps(first_point, W1, b1, W2, b2, has_b1, has_b2, mmnp=np.float16):
    """Per-core input maps with the device operand layouts."""
    w1r = np.ascontiguousarray(
        W1.reshape(DC, 128, HC, 128).transpose(1, 0, 2, 3).reshape(128, DC * HC * 128)
    ).astype(mmnp)
    w2r = np.ascontiguousarray(
        W2.reshape(HC, 128, DC, 128).transpose(1, 0, 2, 3).reshape(128, HC * DC * 128)
    ).astype(mmnp)
    ident = np.eye(128, dtype=np.float32)
    b1c = np.ascontiguousarray(b1.reshape(HC, 128).T).astype(np.float32)
    b2c = np.ascontiguousarray(b2.reshape(DC, 128).T).astype(np.float32)

    in_maps = []
    for i in range(NCORES):
        shard = first_point[i * BP : (i + 1) * BP]  # [128, 512]
        fpT = np.ascontiguousarray(
            shard.reshape(BP, DC, 128).transpose(2, 1, 0).reshape(128, D)
        )
        m = {
            "w1r": w1r,
            "w2r": w2r,
            "ident": ident,
            "fp32d": fpT.astype(np.float32),
            "fp16d": fpT.astype(mmnp),
        }
        if has_b1:
            m["b1c"] = b1c
        if has_b2:
            m["b2c"] = b2c
        in_maps.append(m)
    return in_maps


def kernel(first_point, time_steps, W1, b1, W2, b2):
    first_point = np.asarray(first_point, dtype=np.float32)
    time_steps = np.asarray(time_steps, dtype=np.float32)
    W1 = np.asarray(W1, dtype=np.float32)
    b1 = np.asarray(b1, dtype=np.float32)
    W2 = np.asarray(W2, dtype=np.float32)
    b2 = np.asarray(b2, dtype=np.float32)

    dts = tuple(float(x) for x in (time_steps[1:] - time_steps[:-1]))
    has_b1 = bool(np.any(b1 != 0.0))
    has_b2 = bool(np.any(b2 != 0.0))

    key = (dts, has_b1, has_b2)
    if key not in _cache:
        _cache[key] = _build_program(list(dts), has_b1, has_b2)
    nc = _cache[key]

    # host-side operand layouts
    # W1 chunk (c,m) at free offset (c*HC+m)*128: w1r[p, (c*HC+m)*128+q] = W1[c*128+p, m*128+q]
    in_maps = _host_in_maps(first_point, W1, b1, W2, b2, has_b1, has_b2)

    res = run_bass_kernel_spmd(
        nc,
        in_maps,
        core_ids=list(range(NCORES)),
        trace=bool(int(os.environ.get("KERNEL_TRACE", "0"))),
    )
    kernel._last_results = res

    out = np.empty((T, B, D), dtype=np.float32)
    out[0] = first_point
    for i in range(NCORES):
        dump = res.results[i]["yout"]  # [nsteps, 128(p), D] feature-major
        ns = dump.shape[0]
        # dump[t, p, c*128+b] = y[b, c*128+p]  ->  [t, b, c*128+p]
        out[1:, i * BP : (i + 1) * BP, :] = (
            dump.reshape(ns, 128, DC, 128).transpose(0, 3, 2, 1).reshape(ns, BP, D)
        )
    return out



# revision 10
# speedup vs baseline: 2.7097x; 2.7097x over previous
"""Trainium2 Bass kernel for DiffeqSolver (fixed-grid RK4 over a tanh-MLP ODE).

reference:
  f(y) = tanh(y @ W1 + b1) @ W2 + b2        y: [B, D], W1: [D, H], W2: [H, D]
  63 RK4 steps over time_steps[64]; output pred_y [T=64, B=1024, D=512].

Strategy:
  - Data-parallel over batch: 8 cores x 128 rows each. No collectives.
  - All-feature-major on device: state y^T with D on partitions (4 chunks of
    128), batch (128) on the free dim. Both matmuls then use the weights as
    the stationary operand (lhsT) directly -- no activation transposes ever.
      h^T[m] = sum_c W1[c,m]^T @ u^T[c]     (32 matmuls, N=128)
      z^T[j] = sum_k W2[k,j]^T @ g^T[k]     (32 matmuls, N=128)
  - Matmul operands in fp16 (1 cycle/row on PE; fp32 would be 4). PSUM
    accumulation and the RK4 state/combines stay fp32. Empirically this
    yields ~1e-4 global relative error on this problem.
  - dt values and weight layouts are specialized on the host per call.
  - The RK4 combine uses an incremental p-chain (p_i = p_{i-1} + w_i dt/6 k_i)
    so the step boundary only waits on the last stage's z.
  - Output is DMA'd feature-major straight from the state tile (contiguous,
    no PE transposes); the host undoes the transpose when assembling pred_y.
"""

import os
import sys

import numpy as np

if "/opt/trn_rl_repo" not in sys.path:
    sys.path.insert(0, "/opt/trn_rl_repo")

import concourse.bass as bass
import concourse.mybir as mybir
import concourse.tile as tile
from concourse import bacc
from concourse.bass_utils import run_bass_kernel_spmd

B, D, H, T = 1024, 512, 1024, 64
NCORES = 8
BP = B // NCORES          # 128 batch rows per core
DC = D // 128             # 4 D-chunks
HC = H // 128             # 8 H-chunks
NSTEP = T - 1

F32 = mybir.dt.float32
F16 = mybir.dt.float16
F8 = mybir.dt.float8e4

# fp8 path: weights are pre-scaled by a power of two on the host so the
# bulk of the distribution sits in e4m3 normal range (min normal 2^-6;
# unscaled W1/W2 entries have std 1/sqrt(D) ~ 0.03-0.04). The /scale is
# folded into the tanh's pre-scale (MM1) and the RK4 stage constants
# (MM2) -- zero runtime cost.
W1SCALE = 64.0
W2SCALE = 64.0

# "f8": e4m3 + DoubleRow matmuls (2 K-rows/cycle). "f16": original path.
MM_MODE = os.environ.get("MM_MODE", "f8")


def _build_program(dts, has_b1, has_b2, mm_dtype=None, compile=True, reps=1, timing=False, ablate=(), fm_out=True):
    """Trace + compile the per-core SPMD program. dts: list of python floats.

    timing=True: outputs go to internal DRAM (not transferred) and the body
    repeats `reps` times in a HW loop -- for differential wall-clock timing."""
    if mm_dtype is None:
        mm_dtype = F8 if MM_MODE == "f8" else F16
    DR = mm_dtype == F8  # fp8 e4m3 + DoubleRow (2 contraction rows/cycle)
    nsteps = len(dts)
    nc = bacc.Bacc(
        "TRN2",
        target_bir_lowering=False,
        debug=False,
        enable_asserts=True,
        num_devices=NCORES,
    )

    # weight layout: [partition q, k-chunk, outchunk*128+col]; the DoubleRow
    # path slices adjacent k-chunk PAIRS into [128, 2, 128] stationary APs.
    w1r = nc.dram_tensor("w1r", [128, DC, HC * 128], mm_dtype, kind="ExternalInput")
    w2r = nc.dram_tensor("w2r", [128, HC, DC * 128], mm_dtype, kind="ExternalInput")
    ident = nc.dram_tensor("ident", [128, 128], F32, kind="ExternalInput")
    fp32d = nc.dram_tensor("fp32d", [128, D], F32, kind="ExternalInput")
    fp16d = nc.dram_tensor("fp16d", [128, D], mm_dtype, kind="ExternalInput")
    if has_b1:
        b1d = nc.dram_tensor("b1c", [128, HC], F32, kind="ExternalInput")
    if has_b2:
        b2d = nc.dram_tensor("b2c", [128, DC], F32, kind="ExternalInput")
    if timing:
        tout_d = nc.dram_tensor("tout", [128, 4], F32, kind="ExternalOutput")
    else:
        out_d = nc.dram_tensor("yout", [nsteps, 128, D], F32, kind="ExternalOutput")

    AF = mybir.ActivationFunctionType
    OP = mybir.AluOpType

    with tile.TileContext(nc) as tc, tc.tile_pool(name="persist", bufs=1) as persist:
        # ---- persistent tiles -------------------------------------------
        w1sb = persist.tile([128, DC, HC * 128], mm_dtype, tag="w1sb", name="w1sb")
        w2sb = persist.tile([128, HC, DC * 128], mm_dtype, tag="w2sb", name="w2sb")
        idsb = persist.tile([128, 128], F32, tag="idsb", name="idsb")
        yT = persist.tile([128, D], F32, tag="yT", name="yT")      # fp32 state
        u0 = persist.tile([128, D], mm_dtype, tag="u0", name="u0")
        if "tanh" in ablate:
            u0big = persist.tile([128, H], mm_dtype, tag="u0big", name="u0big")
            touch = persist.tile([128, 64], F32, tag="touch", name="touch")
            nc.gpsimd.memset(u0big[:], 0.01)
        elif "dve" in ablate:
            touch = persist.tile([128, 64], F32, tag="touch", name="touch")
        if has_b1:
            b1sb = persist.tile([128, HC], F32, tag="b1sb", name="b1sb")
        if has_b2:
            b2sb = persist.tile([128, DC], F32, tag="b2sb", name="b2sb")

        nc.sync.dma_start(w1sb[:], w1r[:])
        nc.sync.dma_start(w2sb[:], w2r[:])
        nc.sync.dma_start(idsb[:], ident[:])
        nc.sync.dma_start(yT[:], fp32d[:])
        nc.sync.dma_start(u0[:], fp16d[:])
        if has_b1:
            nc.sync.dma_start(b1sb[:], b1d[:])
        if has_b2:
            nc.sync.dma_start(b2sb[:], b2d[:])

        with (
            tc.tile_pool(name="dram", bufs=1, space="DRAM") as dram_pool,
            tc.tile_pool(name="hps", bufs=2, space="PSUM") as hps_pool,
            tc.tile_pool(name="zps", bufs=4 if "zps4" in ablate else 3, space="PSUM") as zps_pool,
            tc.tile_pool(name="ybm", bufs=1, space="PSUM") as ybm_pool,
            tc.tile_pool(name="upool", bufs=3 if "sbuf3" in ablate else 2) as upool,
            tc.tile_pool(name="ppool", bufs=3 if "sbuf3" in ablate else 2) as ppool,
            tc.tile_pool(name="gpool", bufs=3 if "sbuf3" in ablate else 2) as gpool,
            tc.tile_pool(name="kts", bufs=2) as ktpool,
            tc.tile_pool(name="yo", bufs=3) as yopool,
        ):
            def w1chunk(c, m):
                return w1sb[:, c, m * 128 : (m + 1) * 128]

            def w2chunk(k, j):
                return w2sb[:, k, j * 128 : (j + 1) * 128]

            DRMODE = mybir.MatmulPerfMode.DoubleRow

            def f_eval(u16):
                """u16: fp16/fp8 [128, D] feature-major eval point.
                Returns zT psum tile [128, D] fp32 (feature-major; fp8 path:
                scaled by W2SCALE, and = f(u)-b2 when b2 present)."""
                hps = hps_pool.tile([128, H], F32, tag="hps")
                if DR:
                    for m in range(HC):
                        om = hps[:, m * 128 : (m + 1) * 128]
                        for p in range(DC // 2):
                            nc.tensor.matmul(
                                om,
                                w1sb[:, 2 * p : 2 * p + 2, m * 128 : (m + 1) * 128],
                                u16[:, 2 * p * 128 : (2 * p + 2) * 128].rearrange(
                                    "q (two b) -> q two b", two=2
                                ),
                                start=(p == 0),
                                stop=(p == DC // 2 - 1),
                                perf_mode=DRMODE,
                            )
                else:
                    for m in range(HC):
                        om = hps[:, m * 128 : (m + 1) * 128]
                        for c in range(DC):
                            nc.tensor.matmul(
                                om,
                                w1chunk(c, m),
                                u16[:, c * 128 : (c + 1) * 128],
                                start=(c == 0),
                                stop=(c == DC - 1),
                            )
                gt = gpool.tile([128, H], mm_dtype, tag="gt")
                ascale = (1.0 / W1SCALE) if DR else 1.0
                if "tanh" in ablate:
                    # timing-ablation: break the MM1->ACT->MM2 dependency; MM2
                    # streams from a static tile; touch hps so tiles release.
                    nc.vector.tensor_copy(touch[:, 0:8], hps[:, 0:1024:128])
                    gt = u0big
                elif has_b1:
                    for m in range(HC):
                        sl = slice(m * 128, (m + 1) * 128)
                        nc.scalar.activation(
                            gt[:, sl], hps[:, sl], AF.Tanh,
                            bias=b1sb[:, m : m + 1], scale=ascale,
                        )
                else:
                    # bank0 whole, bank1 split in two: MM2's last k-chunks
                    # wait on a 256-wide ACT op instead of 512 (A/B-measured
                    # win together with zps bufs=3)
                    nc.scalar.activation(gt[:, :512], hps[:, :512], AF.Tanh, scale=ascale)
                    nc.scalar.activation(gt[:, 512:768], hps[:, 512:768], AF.Tanh, scale=ascale)
                    nc.scalar.activation(gt[:, 768:], hps[:, 768:], AF.Tanh, scale=ascale)
                if "dve" in ablate:
                    # timing-ablation: MM1 of every eval streams from u0
                    # (vector STT chain off the critical path entirely)
                    pass
                zps = zps_pool.tile([128, D], F32, tag="zps")
                if DR:
                    for j in range(DC):
                        oj = zps[:, j * 128 : (j + 1) * 128]
                        for p in range(HC // 2):
                            nc.tensor.matmul(
                                oj,
                                w2sb[:, 2 * p : 2 * p + 2, j * 128 : (j + 1) * 128],
                                gt[:, 2 * p * 128 : (2 * p + 2) * 128].rearrange(
                                    "q (two b) -> q two b", two=2
                                ),
                                start=(p == 0),
                                stop=(p == HC // 2 - 1),
                                perf_mode=DRMODE,
                            )
                else:
                    for j in range(DC):
                        oj = zps[:, j * 128 : (j + 1) * 128]
                        for k in range(HC):
                            nc.tensor.matmul(
                                oj,
                                w2chunk(k, j),
                                gt[:, k * 128 : (k + 1) * 128],
                                start=(k == 0),
                                stop=(k == HC - 1),
                            )
                return zps

            if timing:
                out_d = dram_pool.tile([nsteps, 128, D], F32, name="out_i")

            from contextlib import nullcontext

            def emit_output(t):
                if "output" in ablate:
                    return
                if fm_out:
                    # feature-major dump: contiguous DMA straight from the
                    # state tile; the host undoes the transpose. Saves the PE
                    # transposes + PSUM evacuation entirely.
                    nc.sync.dma_start(out_d[t], yT[:])
                    return
                # batch-major output for step t (reads yT as of end of step t):
                # 4 PE transposes -> PSUM, evacuate, DMA out. Emitted lazily
                # during step t+1 so it never stalls the PE at the boundary.
                ybm = ybm_pool.tile([128, D], F32, tag="ybm")
                for c in range(DC):
                    sl = slice(c * 128, (c + 1) * 128)
                    nc.tensor.transpose(ybm[:, sl], yT[:, sl], idsb[:])
                yo = yopool.tile([128, D], F32, tag="yo")
                nc.scalar.copy(yo[:], ybm[:])
                nc.sync.dma_start(out_d[t], yo[:])

            loop_ctx = tc.For_i(0, reps, 1) if reps > 1 else nullcontext()
            u_cur = u0
            with loop_ctx:
                for t in range(nsteps):
                    dt = dts[t]
                    # RK4: u_{i+1} = y + c_i k_i;  y' = y + dt/6 sum w_i k_i.
                    # Incremental p-chain: p_i = p_{i-1} + (w_i dt/6) k_i with
                    # p_0 = y, so the boundary only waits on the last z.
                    # fp8 path without b2: zps carries W2SCALE*k, so the
                    # stage constants absorb the 1/W2SCALE.
                    zsc = (1.0 / W2SCALE) if (DR and not has_b2) else 1.0
                    stage_c = [dt * 0.5 * zsc, dt * 0.5 * zsc, dt * zsc]
                    pw = [dt / 6.0 * zsc, dt / 3.0 * zsc, dt / 3.0 * zsc, dt / 6.0 * zsc]
                    p_prev = yT
                    for i in range(4):
                        zps = f_eval(u_cur)
                        if "dve" in ablate:
                            nc.vector.tensor_copy(touch[:, 8:12], zps[:, 0:512:128])
                            continue
                        if has_b2:
                            kt = ktpool.tile([128, D], F32, tag="kt")
                            for j in range(DC):
                                sl = slice(j * 128, (j + 1) * 128)
                                if DR:
                                    # true k = zps/W2SCALE + b2 (ACT engine)
                                    nc.scalar.activation(
                                        kt[:, sl], zps[:, sl], AF.Copy,
                                        bias=b2sb[:, j : j + 1], scale=1.0 / W2SCALE,
                                    )
                                else:
                                    nc.vector.tensor_scalar_add(
                                        kt[:, sl], zps[:, sl], b2sb[:, j : j + 1]
                                    )
                            ksrc = kt
                        else:
                            ksrc = zps
                        if i < 3:
                            un = upool.tile([128, D], mm_dtype, tag="un")
                            if "ustt2" in ablate:
                                # first half reads z chunks 0-1 (ready at 50%
                                # of MM2, j-outer) -> runs under MM2's tail;
                                # only the 256-wide second half is exposed.
                                nc.vector.scalar_tensor_tensor(
                                    un[:, 0:256], ksrc[:, 0:256], stage_c[i], yT[:, 0:256], OP.mult, OP.add
                                )
                                nc.vector.scalar_tensor_tensor(
                                    un[:, 256:512], ksrc[:, 256:512], stage_c[i], yT[:, 256:512], OP.mult, OP.add
                                )
                            else:
                                nc.vector.scalar_tensor_tensor(
                                    un[:], ksrc[:], stage_c[i], yT[:], OP.mult, OP.add
                                )
                            u_cur = un
                            pn = ppool.tile([128, D], F32, tag="pn")
                            nc.vector.scalar_tensor_tensor(
                                pn[:], ksrc[:], pw[i], p_prev[:], OP.mult, OP.add
                            )
                            p_prev = pn
                        else:
                            # y_{t+1} = p3 + (dt/6) k4: fp16 for the next
                            # step's first eval point (critical path) first,
                            # then the fp32 state update.
                            if t < nsteps - 1 or timing:
                                un = upool.tile([128, D], mm_dtype, tag="un")
                                if "ustt2" in ablate:
                                    nc.vector.scalar_tensor_tensor(
                                        un[:, 0:256], ksrc[:, 0:256], pw[i], p_prev[:, 0:256], OP.mult, OP.add
                                    )
                                    nc.vector.scalar_tensor_tensor(
                                        un[:, 256:512], ksrc[:, 256:512], pw[i], p_prev[:, 256:512], OP.mult, OP.add
                                    )
                                else:
                                    nc.vector.scalar_tensor_tensor(
                                        un[:], ksrc[:], pw[i], p_prev[:], OP.mult, OP.add
                                    )
                                u_cur = un
                            nc.vector.scalar_tensor_tensor(
                                yT[:], ksrc[:], pw[i], p_prev[:], OP.mult, OP.add
                            )
                        if i == 0 and t > 0:
                            # step t-1's output block, emitted mid-step so the
                            # PE transposes hide behind eval-1 matmuls (yT
                            # still holds y_t here; it's rewritten at i==3).
                            emit_output(t - 1)
                emit_output(nsteps - 1)

            if timing:
                dyo = yopool.tile([128, 4], F32, tag="dyo")
                nc.vector.tensor_copy(dyo[:], yT[:, 0:4])
                nc.sync.dma_start(tout_d[:], dyo[:])

    if compile:
        nc.compile()
    return nc


_cache = {}


def _host_in_maps(first_point, W1, b1, W2, b2, has_b1, has_b2, mmnp=None):
    """Per-core input maps with the device operand layouts."""
    if mmnp is None:
        if MM_MODE == "f8":
            import ml_dtypes

            mmnp = ml_dtypes.float8_e4m3
        else:
            mmnp = np.float16
    wscale = (W1SCALE, W2SCALE) if MM_MODE == "f8" else (1.0, 1.0)
    w1r = np.ascontiguousarray(
        (W1 * wscale[0]).reshape(DC, 128, HC, 128).transpose(1, 0, 2, 3).reshape(128, DC, HC * 128)
    ).astype(mmnp)
    w2r = np.ascontiguousarray(
        (W2 * wscale[1]).reshape(HC, 128, DC, 128).transpose(1, 0, 2, 3).reshape(128, HC, DC * 128)
    ).astype(mmnp)
    ident = np.eye(128, dtype=np.float32)
    b1c = np.ascontiguousarray(b1.reshape(HC, 128).T).astype(np.float32)
    b2c = np.ascontiguousarray(b2.reshape(DC, 128).T).astype(np.float32)

    in_maps = []
    for i in range(NCORES):
        shard = first_point[i * BP : (i + 1) * BP]  # [128, 512]
        fpT = np.ascontiguousarray(
            shard.reshape(BP, DC, 128).transpose(2, 1, 0).reshape(128, D)
        )
        m = {
            "w1r": w1r,
            "w2r": w2r,
            "ident": ident,
            "fp32d": fpT.astype(np.float32),
            "fp16d": fpT.astype(mmnp),
        }
        if has_b1:
            m["b1c"] = b1c
        if has_b2:
            m["b2c"] = b2c
        in_maps.append(m)
    return in_maps


def kernel(first_point, time_steps, W1, b1, W2, b2):
    first_point = np.asarray(first_point, dtype=np.float32)
    time_steps = np.asarray(time_steps, dtype=np.float32)
    W1 = np.asarray(W1, dtype=np.float32)
    b1 = np.asarray(b1, dtype=np.float32)
    W2 = np.asarray(W2, dtype=np.float32)
    b2 = np.asarray(b2, dtype=np.float32)

    dts = tuple(float(x) for x in (time_steps[1:] - time_steps[:-1]))
    has_b1 = bool(np.any(b1 != 0.0))
    has_b2 = bool(np.any(b2 != 0.0))

    key = (dts, has_b1, has_b2, MM_MODE)
    if key not in _cache:
        _cache[key] = _build_program(list(dts), has_b1, has_b2)
    nc = _cache[key]

    # host-side operand layouts
    # W1 chunk (c,m) at free offset (c*HC+m)*128: w1r[p, (c*HC+m)*128+q] = W1[c*128+p, m*128+q]
    in_maps = _host_in_maps(first_point, W1, b1, W2, b2, has_b1, has_b2)

    res = run_bass_kernel_spmd(
        nc,
        in_maps,
        core_ids=list(range(NCORES)),
        trace=bool(int(os.environ.get("KERNEL_TRACE", "0"))),
    )
    kernel._last_results = res

    out = np.empty((T, B, D), dtype=np.float32)
    out[0] = first_point
    for i in range(NCORES):
        dump = res.results[i]["yout"]  # [nsteps, 128(p), D] feature-major
        ns = dump.shape[0]
        # dump[t, p, c*128+b] = y[b, c*128+p]  ->  [t, b, c*128+p]
        out[1:, i * BP : (i + 1) * BP, :] = (
            dump.reshape(ns, 128, DC, 128).transpose(0, 3, 2, 1).reshape(ns, BP, D)
        )
    return out

